# revision 15
# baseline (speedup 1.0000x reference)
"""GATv2 GNN (4 layers + head) on 8 trn2 NeuronCores via Bass/Tile.

Sharding: nodes partitioned 1000/core (padded to 1024 rows); edges assigned to
the core owning their destination; weights replicated. Per layer:
  - feature matmuls xla = h @ (Wl * a), xr_a = h @ (Wr * a)   [a folded into W]
  - AllGather of xla shards (fp16) -> per-core DRAM copy of all source rows
  - dma_gather of source/dest rows per edge slot (128 edges per slot)
  - attention scores via sign-split leaky-relu accumulation on ScalarE:
      e = sum_c a_c*LR(u_c) = sum_{a>0} LR(w) - sum_{a<0} LR(-w),  w = a*u
  - softmax without max-shift (exp directly; segment denominators via the
    same one-hot S0 matmuls that aggregate the numerator)
  - numer[d,:] = sum_e S0[d,e]*ex_e*xls_e on TensorE, per 128-dst block
  - BN (train-mode) with cross-core AllReduce of sum/sumsq; BN absorbs the
    a-scaling exactly via sign-folded gamma and per-channel eps*a^2.

Host->device transfer is the wall-clock bottleneck (axon tunnel ~55MB/s):
  - replicated weights are de-replicated: each core uploads a distinct 1/8
    row-slice and full matrices are reassembled on-device with AllGather
  - the one-hot S0 scatter matrix is built on-device (is_equal vs iota)
  - gather index tables are sent in their minimal [16, n/16] form and
    replicated to 128 partitions on-device
  - all small constants ride in one packed [128, C] f32 array
  - the feature/gather data path uses fp16 (same bytes as bf16, ~4x less
    rounding error); the exp/softmax path stays bf16 (raw exp(e) spans
    ~1e-28..3e2 on this data, below fp16's subnormal floor)
  - h0 ships in 12-bit fixed point (1.5 B/elem): q = round(h0/s) in
    [-2047,2047] sent as an int8 plane A = q>>4 plus a packed lo-nibble
    plane. On-device bitwise masking splits A into two fp8e4m3-exact
    integer planes ((A&0xF0) in multiples of 16, (A&0x0F) in [0,15]) and
    the layer-0 matmul runs three mixed fp8(x) x fp16(16W / W) passes per
    k-chunk accumulating in the same PSUM group, so reconstruction is
    exact and the only loss is the 12-bit quantization itself.
  - weights fold only sign(a); |a| is applied on-device (pabs broadcast)
    before the Prelu score pass, keeping fp16/fp8 weight magnitudes
    uniform (a-folded columns span 5 decades and go subnormal).
"""

import os
import sys
from contextlib import ExitStack

import numpy as np
import ml_dtypes

sys.path.insert(0, "/opt/trn_rl_repo")

import concourse.bass as bass  # noqa: E402
import concourse.tile as tile  # noqa: E402
from concourse import bacc, mybir  # noqa: E402
from concourse.bass_utils import run_bass_kernel_spmd  # noqa: E402

NC = 8
N = 8000
NPC = 1000
ROWS = 1024
F_IN = 3201
F_PAD = 3328  # 26 * 128
GS = 4        # slots per dma_gather group (all layers)
BF = ml_dtypes.bfloat16
F16 = np.float16

# (Cin_pad, Cout, H, Cc)
LAYERS = [(F_PAD, 1024, 2, 512), (1024, 512, 1, 512),
          (512, 512, 1, 512), (512, 512, 1, 512)]

# packed-constants column offsets (cpack [128, CPACK_BASE + SLOTS] f32)
O_WHP, O_IVD, O_DMY, O_SC4, O_B4P = 0, 8, 16, 24, 28
O_BN = {0: 32, 1: 56, 2: 68}   # bn li: 3 groups of nch cols (g, b, eps)
O_EYE, O_IOTA, O_DSTF = 80, 208, 336
CPACK_BASE = 336


def _groups(slots, gs):
    g, s = [], 0
    while s < slots:
        g.append((s, min(gs, slots - s)))
        s += min(gs, slots - s)
    return g


def _wrap_idx(idx_flat, slots, gsz=GS):
    """Pack a flat idx list into [16, n/16] column-major-16 wrapped layout,
    independently per dma_gather group (gsz slots each). The 8x partition
    replication the DMA needs is done on-device."""
    cols = []
    for g0, gs in _groups(slots, gsz):
        part = idx_flat[g0 * 128:(g0 + gs) * 128]
        cols.append(np.ascontiguousarray(part.reshape(-1, 16).T))
    return np.concatenate(cols, axis=1).astype(np.int16)


def build_structs(edge_index):
    src = np.concatenate([edge_index[0], np.arange(N)]).astype(np.int64)
    dst = np.concatenate([edge_index[1], np.arange(N)]).astype(np.int64)
    deg = np.bincount(dst, minlength=N).astype(np.float32)

    core_of = dst // NPC
    dst_local = dst % NPC
    blk = dst_local // 128
    lists = [[np.nonzero((core_of == c) & (blk == b))[0] for b in range(8)]
             for c in range(NC)]
    S = [max(int(np.ceil(len(lists[c][b]) / 128)) for c in range(NC))
         for b in range(8)]
    off = np.concatenate([[0], np.cumsum(S)]).astype(int)
    SLOTS = int(off[-1])

    src_pos = np.zeros((NC, SLOTS * 128), np.int16)
    dst_pos = np.zeros((NC, SLOTS * 128), np.int16)
    # dst-within-block for on-device one-hot build; -1 marks padding slots
    # (is_equal never fires -> zero row, matching a host-built S0)
    dstf = np.full((NC, 128, SLOTS), -1.0, np.float32)
    for c in range(NC):
        for b in range(8):
            e = lists[c][b]
            e = e[np.lexsort((src[e], dst[e]))]
            L = off[b] * 128 + np.arange(len(e))
            src_pos[c, L] = ((src[e] // NPC) * ROWS + (src[e] % NPC)).astype(np.int16)
            dst_pos[c, L] = dst_local[e].astype(np.int16)
            dstf[c, L % 128, L // 128] = (dst_local[e] - b * 128).astype(np.float32)
    blk_of_slot = np.concatenate([[b] * S[b] for b in range(8)]).astype(int)
    return dict(deg=deg, S=S, off=off, SLOTS=SLOTS, src_pos=src_pos,
                dst_pos=dst_pos, dstf=dstf, blk_of_slot=blk_of_slot)


def prep_weights(inputs):
    """Sign-sort channels per head, fold a into W columns, fold sign(a) and
    eps*a^2 into BN; permute consumer rows."""
    out = {}
    prev_perm = None
    npos_all = []
    for li, (cin, cout, H, Cc) in enumerate(LAYERS):
        wl = np.asarray(inputs[f"W{li + 1}l"]).astype(np.float64)
        wr = np.asarray(inputs[f"W{li + 1}r"]).astype(np.float64)
        a = np.asarray(inputs[f"a{li + 1}"]).reshape(H, Cc).astype(np.float64)
        if prev_perm is not None:
            wl = wl[prev_perm]
            wr = wr[prev_perm]
        perm = np.zeros(H * Cc, int)
        npos = []
        for h in range(H):
            ph = np.argsort(~(a[h] > 0), kind="stable")
            perm[h * Cc:(h + 1) * Cc] = h * Cc + ph
            npos.append(int((a[h] > 0).sum()))
        npos_all.append(npos)
        a_s = a.reshape(-1)[perm]
        # fold only sign(a) into W (keeps fp16 weight magnitudes uniform);
        # |a| is applied on-device to the gathered edge features before the
        # Prelu score accumulation (pabs broadcast row).
        wl = wl[:, perm] * np.sign(a_s)[None, :]
        wr = wr[:, perm] * np.sign(a_s)[None, :]
        out[f"wl{li}"] = wl.astype(np.float32)
        out[f"wr{li}"] = wr.astype(np.float32)
        out[f"pabs{li}"] = np.abs(a_s).astype(np.float32)
        if li < 3:
            g = np.asarray(inputs[f"bn{li + 1}_g"])[perm] * np.sign(a_s)
            b = np.asarray(inputs[f"bn{li + 1}_b"])[perm]
            eps = np.full(len(a_s), 1e-5)
            out[f"bn{li}"] = (g.astype(np.float32), b.astype(np.float32),
                              eps.astype(np.float32))
        else:
            out["scale4"] = np.sign(a_s).astype(np.float32)
            out["bias4"] = np.asarray(inputs["b4"])[perm].astype(np.float32)
        prev_perm = perm
    out["wh"] = np.asarray(inputs["Wh"])[prev_perm].astype(np.float32)
    out["npos"] = npos_all
    return out


def _pack_pp(vec):
    """[k*128] -> [128, k] per-partition packing (chunk c in column c)."""
    k = len(vec) // 128
    return np.ascontiguousarray(vec.reshape(k, 128).T).astype(np.float32)


_PROGRAM_CACHE = {}


KSTAGES = int(os.environ.get("KSTAGES", "99"))

# per-core row-slab heights of the AllGathered weight matrices
WSLAB = {0: F_PAD // NC, 1: 1024 // NC, 2: 512 // NC, 3: 512 // NC}


def build_program(G, npos, bh_val, h0_scale):
    key = (tuple(G["S"]), tuple(tuple(x) for x in npos), float(bh_val),
           float(h0_scale), KSTAGES)
    if key in _PROGRAM_CACHE:
        return _PROGRAM_CACHE[key]

    SLOTS = G["SLOTS"]
    off = G["off"]
    blk_of_slot = G["blk_of_slot"]
    f32, f32r, bf16, f16, i16 = (mybir.dt.float32, mybir.dt.float32r,
                                 mybir.dt.bfloat16, mybir.dt.float16,
                                 mybir.dt.int16)
    fp8 = mybir.dt.float8e4
    i8, u8 = mybir.dt.int8, mybir.dt.uint8
    AF = mybir.ActivationFunctionType
    ALU = mybir.AluOpType
    CPCK = CPACK_BASE + SLOTS

    nc = bacc.Bacc("TRN2", target_bir_lowering=False, debug=False,
                   num_devices=NC)

    # ---------------- inputs
    # h0 in 12-bit fixed point: q = round(h0/s) in [-2047,2047], q = 16*A + C.
    # h0a holds A (int8); h0b packs C lo-nibbles of node j (low) and j+500
    # (high). On-device these become three exact fp8 matmul planes.
    h0a_d = nc.dram_tensor("h0a", [F_IN, NPC], mybir.dt.int8,
                           kind="ExternalInput")
    h0b_d = nc.dram_tensor("h0b", [F_IN, NPC // 2], mybir.dt.uint8,
                           kind="ExternalInput")
    # weight slabs: each core uploads rows [c*slab:(c+1)*slab] of the
    # [cin, 2*cout] (l|r)-concatenated matrix; full matrices reassembled
    # on-device via AllGather. w1/w2/w3 slabs ride in one f32 input.
    w0s_d = nc.dram_tensor("w0s", [WSLAB[0], 2048], f16, kind="ExternalInput")
    w123_d = nc.dram_tensor("w123s", [256, 1024], f16, kind="ExternalInput")
    idx_d = nc.dram_tensor("idx", [16, SLOTS * 32], i16, kind="ExternalInput")
    cpk_d = nc.dram_tensor("cpack", [128, CPCK], f32, kind="ExternalInput")
    prow_d = nc.dram_tensor("prow", [1, 2560], f32, kind="ExternalInput")
    pred_d = nc.dram_tensor("pred", [1, ROWS], f32, kind="ExternalOutput")

    with tile.TileContext(nc) as tc, ExitStack() as top:
        dram = top.enter_context(tc.tile_pool(name="dram", bufs=1, space="DRAM"))
        const_p = top.enter_context(tc.tile_pool(name="const", bufs=1))
        s0_p = top.enter_context(tc.tile_pool(name="s0p", bufs=1))

        # -------- reassemble replicated weights on-device (AllGather) -----
        # collectives cannot read IO tensors: stage each slab into an
        # Internal DRAM tile first (HBM->HBM DMA), then AllGather.
        wfull = {
            0: dram.tile([F_PAD, 2048], f16, tag="w0f", name="w0f"),
            1: dram.tile([1024, 1024], f16, tag="w1f", name="w1f"),
            2: dram.tile([512, 1024], f16, tag="w2f", name="w2f"),
            3: dram.tile([512, 1024], f16, tag="w3f", name="w3f"),
        }
        wstage = {
            0: dram.tile([WSLAB[0], 2048], f16, tag="w0st", name="w0st"),
            1: dram.tile([WSLAB[1], 1024], f16, tag="w1st", name="w1st"),
            2: dram.tile([WSLAB[2], 1024], f16, tag="w2st", name="w2st"),
            3: dram.tile([WSLAB[3], 1024], f16, tag="w3st", name="w3st"),
        }
        nc.sync.dma_start(wstage[0][:], w0s_d[:])
        nc.sync.dma_start(wstage[1][:], w123_d[0:128, :])
        nc.sync.dma_start(wstage[2][:], w123_d[128:192, :])
        nc.sync.dma_start(wstage[3][:], w123_d[192:256, :])
        for li in range(4):
            nc.gpsimd.collective_compute(
                "AllGather", ALU.bypass,
                replica_groups=[list(range(NC))],
                ins=[wstage[li][:].opt()],
                outs=[wfull[li][:].opt()],
            )

        # -------- packed small constants + on-device index replication ----
        cp = const_p.tile([128, CPCK], f32, tag="cpack", name="cpack")
        nc.sync.dma_start(cp[:], cpk_d[:])
        # K=2 stationary of 0.5s for the |a| row broadcast (fp32 matmul
        # rejects K=1); prow duplicated into both partitions.
        prow = const_p.tile([2, 2560], f32, tag="prow", name="prow")
        nc.sync.dma_start(prow[0:1, :], prow_d[:])
        nc.sync.dma_start(prow[1:2, :], prow_d[:])
        ones1 = const_p.tile([2, 128], f32, tag="ones1", name="ones1")
        nc.gpsimd.memset(ones1[:], 0.5)
        isrc = const_p.tile([128, SLOTS * 8], i16, tag="isrc", name="isrc")
        idst = const_p.tile([128, SLOTS * 8], i16, tag="idst", name="idst")
        isrc8 = const_p.tile([128, SLOTS * 8], i16, tag="isrc8", name="isrc8")
        idst8 = const_p.tile([128, SLOTS * 8], i16, tag="idst8", name="idst8")
        for r in range(8):
            nc.sync.dma_start(isrc[r * 16:(r + 1) * 16, :],
                              idx_d[:, 0:SLOTS * 8])
            nc.sync.dma_start(idst[r * 16:(r + 1) * 16, :],
                              idx_d[:, SLOTS * 8:SLOTS * 16])
            nc.sync.dma_start(isrc8[r * 16:(r + 1) * 16, :],
                              idx_d[:, SLOTS * 16:SLOTS * 24])
            nc.sync.dma_start(idst8[r * 16:(r + 1) * 16, :],
                              idx_d[:, SLOTS * 24:SLOTS * 32])
        eye = cp[:, O_EYE:O_EYE + 128]

        # -------- build one-hot S0 on-device: S0[p, s*128+d] = (dstf[p,s]==d)
        s0_sb = s0_p.tile([128, SLOTS * 128], bf16)
        for s in range(SLOTS):
            nc.vector.tensor_scalar(
                s0_sb[:, s * 128:(s + 1) * 128], cp[:, O_IOTA:O_IOTA + 128],
                cp[:, O_DSTF + s:O_DSTF + s + 1], None, op0=ALU.is_equal)

        xla_sh, xla_full, xr_loc = {}, {}, {}
        for li, (_, cout, _, _) in enumerate(LAYERS):
            xla_sh[li] = dram.tile([ROWS, cout], f16, tag=f"xlash{li}", name=f"xlash{li}")
            xla_full[li] = dram.tile([NC * ROWS, cout], f16, tag=f"xlaf{li}", name=f"xlaf{li}")
            xr_loc[li] = dram.tile([ROWS, cout], f16, tag=f"xrloc{li}", name=f"xrloc{li}")

        # hT pools managed non-nested (layer li's hT dies after its F phase)
        # layer 0: reconstruct q = 16*A + C as ONE f16 plane per k-chunk --
        # f16 holds integers up to 2048 exactly, so the 12-bit value is
        # exact and layer 0 keeps the plain 2-matmul structure. The s scale
        # rides on the psum copy-out activation.
        hT_pool = {0: tc.alloc_tile_pool(name="hT0", bufs=1)}
        hT = []
        with tc.tile_pool(name="h0stg", bufs=3) as stg:
            for k in range(F_PAD // 128):
                t = hT_pool[0].tile([128, ROWS], f16, tag=f"h{k}",
                                    name=f"hT0_{k}")
                nc.gpsimd.memset(t[:, NPC:ROWS], 0.0)
                a8 = stg.tile([128, NPC], i8, tag="a8")
                b8 = stg.tile([128, NPC // 2], u8, tag="b8")
                if k < 25:
                    nc.sync.dma_start(a8[:], h0a_d[k * 128:(k + 1) * 128, :])
                    nc.sync.dma_start(b8[:], h0b_d[k * 128:(k + 1) * 128, :])
                else:
                    nc.gpsimd.memset(a8[:], 0)
                    nc.gpsimd.memset(b8[:], 0)
                    nc.sync.dma_start(a8[0:1, :], h0a_d[3200:3201, :])
                    nc.sync.dma_start(b8[0:1, :], h0b_d[3200:3201, :])
                af = stg.tile([128, NPC], f32, tag="af")
                nc.vector.tensor_copy(af[:], a8[:])
                a16 = stg.tile([128, NPC], f32, tag="a16")
                nc.vector.tensor_scalar_mul(a16[:], af[:], 16.0)
                bl8 = stg.tile([128, NPC // 2], u8, tag="bl8")
                nc.vector.tensor_scalar(bl8[:], b8[:], 15, None,
                                        op0=ALU.bitwise_and)
                blf = stg.tile([128, NPC // 2], f32, tag="blf")
                nc.vector.tensor_copy(blf[:], bl8[:])
                bf = stg.tile([128, NPC // 2], f32, tag="bf")
                nc.vector.tensor_copy(bf[:], b8[:])
                bh_ = stg.tile([128, NPC // 2], f32, tag="bh_")
                nc.vector.tensor_tensor(bh_[:], bf[:], blf[:],
                                        op=ALU.subtract)
                chi = stg.tile([128, NPC // 2], f32, tag="chi")
                nc.vector.tensor_scalar_mul(chi[:], bh_[:], 1.0 / 16.0)
                nc.vector.tensor_tensor(t[:, 0:NPC // 2], a16[:, 0:NPC // 2],
                                        blf[:], op=ALU.add)
                nc.vector.tensor_tensor(t[:, NPC // 2:NPC],
                                        a16[:, NPC // 2:NPC], chi[:],
                                        op=ALU.add)
                hT.append(t)

        for li, (cin, cout, H, Cc) in enumerate(LAYERS):
            kc = cin // 128
            nch_out = cout // 128
            wcat = wfull[li][:].rearrange("(k p) n -> k p n", p=128)
            if 4 * li + 0 >= KSTAGES:
                break

            # ================= feature phase =================
            with ExitStack() as lf:
                fpsum = lf.enter_context(
                    tc.tile_pool(name=f"fps{li}", bufs=1 if li == 0 else 2,
                                 space="PSUM"))
                fout = lf.enter_context(tc.tile_pool(name=f"fo{li}", bufs=4))
                wpool = lf.enter_context(tc.tile_pool(name=f"w{li}", bufs=1))
                wsp = lf.enter_context(tc.tile_pool(name=f"ws{li}", bufs=8))

                if li == 0:
                    # W streamed: for each n-half and m-group of 4, stream K.
                    # lhsT holds integer q = 16A+C exactly in f16; xla =
                    # s*(q@W) with s on the psum copy-out activation.
                    for nh in range(2):
                        nsl = slice(nh * 512, (nh + 1) * 512)
                        nsr = slice(1024 + nh * 512, 1024 + (nh + 1) * 512)
                        for mg in range(2):
                            psl = [fpsum.tile([128, 512], f32, tag=f"psl{j}", name=f"psl{j}") for j in range(4)]
                            psr = [fpsum.tile([128, 512], f32, tag=f"psr{j}", name=f"psr{j}") for j in range(4)]
                            for k in range(kc):
                                tl = wsp.tile([128, 512], f16, tag="wls")
                                nc.sync.dma_start(tl[:], wcat[k, :, nsl])
                                tr = wsp.tile([128, 512], f16, tag="wrs")
                                nc.sync.dma_start(tr[:], wcat[k, :, nsr])
                                st, sp0 = k == 0, k == kc - 1
                                for j in range(4):
                                    m = mg * 4 + j
                                    msl = slice(m * 128, (m + 1) * 128)
                                    nc.tensor.matmul(psl[j][:],
                                                     hT[k][:, msl], tl[:],
                                                     start=st, stop=sp0)
                                    nc.tensor.matmul(psr[j][:],
                                                     hT[k][:, msl], tr[:],
                                                     start=st, stop=sp0)
                            for j in range(4):
                                m = mg * 4 + j
                                rsl = slice(m * 128, (m + 1) * 128)
                                xla_m = fout.tile([128, 512], f16, tag="xlam")
                                nc.scalar.activation(xla_m[:], psl[j][:],
                                                     AF.Copy,
                                                     scale=float(h0_scale))
                                nc.sync.dma_start(xla_sh[li][rsl, nsl], xla_m[:])
                                xr_m = fout.tile([128, 512], f16, tag="xrm")
                                nc.scalar.activation(xr_m[:], psr[j][:],
                                                     AF.Copy,
                                                     scale=float(h0_scale))
                                nc.sync.dma_start(xr_loc[li][rsl, nsl], xr_m[:])
                else:
                    wl_t, wr_t = [], []
                    for k in range(kc):
                        tl = wpool.tile([128, cout], f16, tag=f"wl{k}")
                        tr = wpool.tile([128, cout], f16, tag=f"wr{k}")
                        nc.gpsimd.dma_start(tl[:], wcat[k, :, 0:cout])
                        nc.gpsimd.dma_start(tr[:], wcat[k, :, cout:2 * cout])
                        wl_t.append(tl)
                        wr_t.append(tr)
                    for m in range(8):
                        psl = fpsum.tile([128, cout], f32, tag="psl")
                        psr = fpsum.tile([128, cout], f32, tag="psr")
                        for k in range(kc):
                            lhsT = hT[k][:, m * 128:(m + 1) * 128]
                            st, sp0 = k == 0, k == kc - 1
                            nc.tensor.matmul(psl[:], lhsT, wl_t[k][:],
                                             start=st, stop=sp0)
                            nc.tensor.matmul(psr[:], lhsT, wr_t[k][:],
                                             start=st, stop=sp0)
                        rsl = slice(m * 128, (m + 1) * 128)
                        xla_m = fout.tile([128, cout], f16, tag="xlam")
                        nc.scalar.activation(xla_m[:], psl[:], AF.Copy)
                        nc.sync.dma_start(xla_sh[li][rsl, :], xla_m[:])
                        xr_m = fout.tile([128, cout], f16, tag="xrm")
                        nc.scalar.activation(xr_m[:], psr[:], AF.Copy)
                        nc.sync.dma_start(xr_loc[li][rsl, :], xr_m[:])

            hT_pool[li].release()  # free this layer's hT
            nch_out_ = cout // 128
            hT_pool[li + 1] = tc.alloc_tile_pool(name=f"hT{li + 1}", bufs=1)
            hT_next = [hT_pool[li + 1].tile([128, ROWS], f16, tag=f"h{c}",
                                            name=f"hT{li + 1}_{c}")
                       for c in range(nch_out_)]

            if 4 * li + 1 >= KSTAGES:
                break
            nc.gpsimd.collective_compute(
                "AllGather", ALU.bypass,
                replica_groups=[list(range(NC))],
                ins=[xla_sh[li][:].opt()],
                outs=[xla_full[li][:].opt()],
            )
            if 4 * li + 2 >= KSTAGES:
                break

            # ================= edge phase =================
            aggp = tc.alloc_tile_pool(name=f"agg{li}", bufs=1)
            agg_full = aggp.tile([128, 8, cout], f32, tag="agg")
            # broadcast |a| row to all 128 partitions via K=1 outer product
            pbc = aggp.tile([128, cout], f32, tag="pbc")
            PO = {0: 0, 1: 1024, 2: 1536, 3: 2048}[li]
            with tc.tile_pool(name=f"pb{li}", bufs=2, space="PSUM") as pbp:
                for n in range(cout // 512):
                    pps = pbp.tile([128, 512], f32, tag="pps")
                    nc.tensor.matmul(
                        pps[:], ones1[:],
                        prow[0:2, PO + n * 512:PO + (n + 1) * 512],
                        start=True, stop=True)
                    nc.scalar.activation(pbc[:, n * 512:(n + 1) * 512],
                                         pps[:], AF.Copy)
            with ExitStack() as le:
                gp = le.enter_context(tc.tile_pool(name=f"g{li}", bufs=3))
                wp = le.enter_context(tc.tile_pool(name=f"wt{li}", bufs=2))
                sp_ = le.enter_context(tc.tile_pool(name=f"sm{li}", bufs=4))
                scp = le.enter_context(tc.tile_pool(name=f"scr{li}", bufs=8))
                epsum = le.enter_context(
                    tc.tile_pool(name=f"eps{li}", bufs=2, space="PSUM"))

                numer_ps = denom_ps = None
                GSL = GS if li == 0 else 8
                isrc_t = isrc if li == 0 else isrc8
                idst_t = idst if li == 0 else idst8
                for g0, gs in _groups(SLOTS, GSL):
                    xls = gp.tile([128, GSL, cout], f16, tag="xls")
                    nc.gpsimd.dma_gather(
                        xls[:, 0:gs, :], xla_full[li][:],
                        isrc_t[:, g0 * 8:(g0 + gs) * 8], gs * 128, gs * 128, cout)
                    xrg = gp.tile([128, GSL, cout], f16, tag="xrg")
                    nc.gpsimd.dma_gather(
                        xrg[:, 0:gs, :], xr_loc[li][:],
                        idst_t[:, g0 * 8:(g0 + gs) * 8], gs * 128, gs * 128, cout)
                    wt = wp.tile([128, GSL, cout], f16, tag="wt")
                    nc.vector.tensor_add(wt[:, 0:gs, :], xls[:, 0:gs, :],
                                         xrg[:, 0:gs, :])
                    # wtp = |a| * (sign-folded u) = a*u, per channel
                    wtp = wp.tile([128, GSL, cout], f16, tag="wtp")
                    for si in range(gs):
                        nc.vector.tensor_mul(wtp[:, si, :], wt[:, si, :],
                                             pbc[:])
                    pq = sp_.tile([128, GSL, H, 2], f32, tag="pq")
                    for si in range(gs):
                        for h in range(H):
                            b0 = h * Cc
                            nph = npos[li][h]
                            # evaluate LR at 16x scale (LUT abs-error there
                            # is cheaper); 1/16 folded into the Exp scale
                            scr = scp.tile([128, 512], bf16, tag="scr")
                            nc.scalar.activation(
                                scr[:, 0:nph], wtp[:, si, b0:b0 + nph],
                                AF.Prelu, scale=16.0, alpha=0.2,
                                accum_out=pq[:, si, h, 0:1])
                            scr2 = scp.tile([128, 512], bf16, tag="scr")
                            nc.scalar.activation(
                                scr2[:, 0:Cc - nph], wtp[:, si, b0 + nph:b0 + Cc],
                                AF.Prelu, scale=-16.0, alpha=0.2,
                                accum_out=pq[:, si, h, 1:2])
                    esc = sp_.tile([128, GSL, H], f32, tag="esc")
                    nc.vector.tensor_tensor(
                        esc[:, 0:gs, :], pq[:, 0:gs, :, 0], pq[:, 0:gs, :, 1],
                        op=ALU.subtract)
                    exf = sp_.tile([128, GSL, H], f32, tag="exf")
                    nc.scalar.activation(exf[:, 0:gs, :], esc[:, 0:gs, :], AF.Exp,
                                         scale=1.0 / 16.0)
                    exb = sp_.tile([128, GSL, H], bf16, tag="exb")
                    nc.vector.tensor_copy(exb[:, 0:gs, :], exf[:, 0:gs, :])
                    # round the numerator scalar through the SAME bf16 values
                    # the denominator matmul uses, so rounding cancels in the
                    # softmax ratio (ts scalars must be f32)
                    exf2 = sp_.tile([128, GSL, H], f32, tag="exf2")
                    nc.vector.tensor_copy(exf2[:, 0:gs, :], exb[:, 0:gs, :])
                    y = wp.tile([128, GSL, cout], bf16, tag="y")
                    for si in range(gs):
                        for h in range(H):
                            nc.vector.tensor_scalar_mul(
                                y[:, si, h * Cc:(h + 1) * Cc],
                                xls[:, si, h * Cc:(h + 1) * Cc],
                                exf2[:, si, h:h + 1])
                    for si in range(gs):
                        sg = g0 + si
                        b = int(blk_of_slot[sg])
                        first = sg == off[b]
                        last = sg == off[b + 1] - 1
                        if first:
                            numer_ps = epsum.tile([128, cout], f32, tag="nps")
                            denom_ps = epsum.tile([128, H], f32, tag="dps")
                        lhsT = s0_sb[:, sg * 128:(sg + 1) * 128]
                        for n in range(cout // 512):
                            sl = slice(n * 512, (n + 1) * 512)
                            nc.tensor.matmul(numer_ps[:, sl], lhsT, y[:, si, sl],
                                             start=first, stop=last)
                        nc.tensor.matmul(denom_ps[:], lhsT, exb[:, si, :],
                                         start=first, stop=last)
                        if last:
                            dn = sp_.tile([128, H], f32, tag="dn")
                            rec = sp_.tile([128, H], f32, tag="rec")
                            c1 = sp_.tile([128, H], f32, tag="c1")
                            for h in range(H):
                                nc.vector.tensor_add(
                                    dn[:, h:h + 1], denom_ps[:, h:h + 1],
                                    cp[:, O_DMY + b:O_DMY + b + 1])
                            nc.vector.reciprocal(rec[:], dn[:])
                            for h in range(H):
                                nc.vector.tensor_mul(
                                    c1[:, h:h + 1], rec[:, h:h + 1],
                                    cp[:, O_IVD + b:O_IVD + b + 1])
                            for h in range(H):
                                nc.vector.tensor_scalar_mul(
                                    agg_full[:, b, h * Cc:(h + 1) * Cc],
                                    numer_ps[:, h * Cc:(h + 1) * Cc],
                                    c1[:, h:h + 1])

            # ================= transpose + BN =================
            if 4 * li + 3 >= KSTAGES:
                aggp.release()
                break
            with ExitStack() as lt:
                tps = lt.enter_context(
                    tc.tile_pool(name=f"tp{li}", bufs=4, space="PSUM"))
                tsp = lt.enter_context(tc.tile_pool(name=f"ts{li}", bufs=3))
                raws = lt.enter_context(tc.tile_pool(name=f"rw{li}", bufs=1))
                raw = ([raws.tile([128, ROWS], f32, tag=f"r{c}", name=f"raw{li}_{c}") for c in range(nch_out)] if li < 3 else None)
                for c in range(nch_out):
                    for b in range(8):
                        pt = tps.tile([128, 128], f32, tag="tp")
                        nc.tensor.transpose(
                            pt[:], agg_full[:, b, c * 128:(c + 1) * 128], eye)
                        if li < 3:
                            nc.scalar.activation(
                                raw[c][:, b * 128:(b + 1) * 128], pt[:], AF.Copy)
                        else:
                            nc.scalar.activation(
                                hT_next[c][:, b * 128:(b + 1) * 128], pt[:],
                                AF.Relu, scale=cp[:, O_SC4 + c:O_SC4 + c + 1],
                                bias=cp[:, O_B4P + c:O_B4P + c + 1])

                if li < 3:
                    stat = tsp.tile([128, 2 * nch_out], f32, tag="stat")
                    for c in range(nch_out):
                        nc.vector.reduce_sum(stat[:, c:c + 1], raw[c][:, 0:NPC],
                                             axis=mybir.AxisListType.X)
                        sq = tsp.tile([128, NPC], f32, tag="sq")
                        nc.scalar.activation(
                            sq[:], raw[c][:, 0:NPC], AF.Square,
                            accum_out=stat[:, nch_out + c:nch_out + c + 1])
                    st_in = dram.tile([128, 2 * nch_out], f32, tag=f"sti{li}")
                    st_out = dram.tile([128, 2 * nch_out], f32, tag=f"sto{li}")
                    nc.sync.dma_start(st_in[:], stat[:])
                    nc.gpsimd.collective_compute(
                        "AllReduce", ALU.add,
                        replica_groups=[list(range(NC))],
                        ins=[st_in[:].opt()], outs=[st_out[:].opt()])
                    gstat = tsp.tile([128, 2 * nch_out], f32, tag="gstat")
                    nc.sync.dma_start(gstat[:], st_out[:])
                    mean = tsp.tile([128, nch_out], f32, tag="mean")
                    nc.scalar.mul(mean[:], gstat[:, 0:nch_out], 1.0 / N)
                    msq = tsp.tile([128, nch_out], f32, tag="msq")
                    nc.scalar.mul(msq[:], gstat[:, nch_out:2 * nch_out], 1.0 / N)
                    m2 = tsp.tile([128, nch_out], f32, tag="m2")
                    nc.vector.tensor_mul(m2[:], mean[:], mean[:])
                    var = tsp.tile([128, nch_out], f32, tag="var")
                    nc.vector.tensor_tensor(var[:], msq[:], m2[:], op=ALU.subtract)
                    ob = O_BN[li]
                    nch = nch_out
                    veps = tsp.tile([128, nch_out], f32, tag="veps")
                    nc.vector.tensor_add(veps[:], var[:],
                                         cp[:, ob + 2 * nch:ob + 3 * nch])
                    sd = tsp.tile([128, nch_out], f32, tag="sd")
                    nc.scalar.activation(sd[:], veps[:], AF.Sqrt)
                    isd = tsp.tile([128, nch_out], f32, tag="isd")
                    nc.vector.reciprocal(isd[:], sd[:])
                    sc = tsp.tile([128, nch_out], f32, tag="sc")
                    nc.vector.tensor_mul(sc[:], isd[:], cp[:, ob:ob + nch])
                    msc = tsp.tile([128, nch_out], f32, tag="msc")
                    nc.vector.tensor_mul(msc[:], mean[:], sc[:])
                    bi = tsp.tile([128, nch_out], f32, tag="bi")
                    nc.vector.tensor_tensor(bi[:], cp[:, ob + nch:ob + 2 * nch],
                                            msc[:], op=ALU.subtract)
                    for c in range(nch_out):
                        nc.scalar.activation(
                            hT_next[c][:], raw[c][:], AF.Relu,
                            scale=sc[:, c:c + 1], bias=bi[:, c:c + 1])
            aggp.release()
            hT = hT_next

        # ================= head =================
        # out[0, n] = sum_c wh[c] * h4T[c, n]; stationary = wh chunk [128, 2]
        # (second column zero to satisfy fp32r even-free-dim), moving = h4T.
        if 16 >= KSTAGES:
            for p in sorted(hT_pool, reverse=True):
                try:
                    hT_pool[p].release()
                except Exception:
                    pass
            with tc.tile_pool(name="zt", bufs=1) as ztp:
                zt = ztp.tile([1, ROWS], f32)
                nc.gpsimd.memset(zt[:], 0.0)
                nc.sync.dma_start(pred_d[:], zt[:])
        else:
          with ExitStack() as lh:
              hps = lh.enter_context(tc.tile_pool(name="hps", bufs=2, space="PSUM"))
              hsb = lh.enter_context(tc.tile_pool(name="hsb", bufs=1))
              ones2 = hsb.tile([128, 2], f32)
              nc.gpsimd.memset(ones2[:], 1.0)
              # t[p, n] = sum_c wh[c*128+p] * h4T[c*128+p, n]  (per-partition)
              acc = hsb.tile([128, ROWS], f32)
              tmp = hsb.tile([128, ROWS], f32)
              nc.vector.tensor_scalar_mul(acc[:], hT[0][:],
                                          cp[:, O_WHP:O_WHP + 1])
              for c in range(1, 4):
                  nc.vector.tensor_scalar_mul(tmp[:], hT[c][:],
                                              cp[:, O_WHP + 2 * c:O_WHP + 2 * c + 1])
                  nc.vector.tensor_add(acc[:], acc[:], tmp[:])
              pred_sb = hsb.tile([1, ROWS], f32)
              for n in range(2):
                  nsl = slice(n * 512, (n + 1) * 512)
                  pp = hps.tile([2, 512], f32, tag="pp")
                  nc.tensor.matmul(pp[:], ones2[:], acc[:, nsl],
                                   start=True, stop=True)
                  nc.scalar.activation(pred_sb[:, nsl], pp[0:1, :], AF.Sigmoid,
                                       bias=float(bh_val))
              nc.sync.dma_start(pred_d[:], pred_sb[:])
          hT_pool[4].release()

    nc.compile()
    _PROGRAM_CACHE[key] = (nc, SLOTS)
    return nc, SLOTS


def _host_prep(inputs):
    x = np.asarray(inputs["x"], np.float32)
    m = x.mean(0)
    v = x.var(0)
    h0 = ((x - m) / np.sqrt(v + 1e-5) * np.asarray(inputs["bn0_g"])
          + np.asarray(inputs["bn0_b"])).astype(np.float32)
    G = build_structs(np.asarray(inputs["edge_index"]))
    W = prep_weights(inputs)
    return h0, G, W


def make_in_maps(h0, G, W):
    SLOTS = G["SLOTS"]
    s = float(np.abs(h0).max() / 2047.0)
    q = np.round(h0 / s).astype(np.int32)          # [-2047, 2047]
    qA = np.floor_divide(q, 16)                    # [-128, 127]
    qC = (q - 16 * qA).astype(np.uint8)            # [0, 15]
    # (l|r)-concatenated weight matrices, sliced into per-core row slabs
    w0cat = np.zeros((F_PAD, 2048), F16)
    w0cat[:F_IN, 0:1024] = W["wl0"]
    w0cat[:F_IN, 1024:2048] = W["wr0"]
    wcat = {0: w0cat}
    for li in (1, 2, 3):
        wcat[li] = np.concatenate([W[f"wl{li}"], W[f"wr{li}"]],
                                  axis=1).astype(F16)
    whp = np.ascontiguousarray(np.stack(
        [W["wh"].reshape(4, 128).T, np.zeros((128, 4), np.float32)],
        axis=2).reshape(128, 8))
    in_maps = []
    for c in range(NC):
        invdeg = np.zeros(ROWS, np.float32)
        invdeg[:NPC] = 1.0 / G["deg"][c * NPC:(c + 1) * NPC]
        dummy = np.zeros(ROWS, np.float32)
        dummy[NPC:] = 1.0
        cpk = np.zeros((128, CPACK_BASE + SLOTS), np.float32)
        cpk[:, O_WHP:O_WHP + 8] = whp
        cpk[:, O_IVD:O_IVD + 8] = _pack_pp(invdeg)
        cpk[:, O_DMY:O_DMY + 8] = _pack_pp(dummy)
        cpk[:, O_SC4:O_SC4 + 4] = _pack_pp(W["scale4"])
        cpk[:, O_B4P:O_B4P + 4] = _pack_pp(W["bias4"])
        for li in (0, 1, 2):
            g, b, e = W[f"bn{li}"]
            nch = 8 if li == 0 else 4
            ob = O_BN[li]
            cpk[:, ob:ob + nch] = _pack_pp(g)
            cpk[:, ob + nch:ob + 2 * nch] = _pack_pp(b)
            cpk[:, ob + 2 * nch:ob + 3 * nch] = _pack_pp(e)
        cpk[:, O_EYE:O_EYE + 128] = np.eye(128, dtype=np.float32)
        cpk[:, O_IOTA:O_IOTA + 128] = np.arange(128, dtype=np.float32)[None, :]
        cpk[:, O_DSTF:O_DSTF + SLOTS] = G["dstf"][c]
        prow = np.zeros((1, 2560), np.float32)
        prow[0, 0:1024] = W["pabs0"]
        prow[0, 1024:1536] = W["pabs1"]
        prow[0, 1536:2048] = W["pabs2"]
        prow[0, 2048:2560] = W["pabs3"]
        A_T = qA[c * NPC:(c + 1) * NPC].T              # [F_IN, NPC]
        C_T = qC[c * NPC:(c + 1) * NPC].T
        m = {
            "h0a": np.ascontiguousarray(A_T.astype(np.int8)),
            "h0b": np.ascontiguousarray(
                (C_T[:, 0:NPC // 2]
                 + 16 * C_T[:, NPC // 2:NPC]).astype(np.uint8)),
            "prow": prow,
            "idx": np.concatenate([_wrap_idx(G["src_pos"][c], SLOTS, 4),
                                   _wrap_idx(G["dst_pos"][c], SLOTS, 4),
                                   _wrap_idx(G["src_pos"][c], SLOTS, 8),
                                   _wrap_idx(G["dst_pos"][c], SLOTS, 8)],
                                  axis=1),
            "cpack": cpk,
            "w0s": np.ascontiguousarray(w0cat[c * WSLAB[0]:(c + 1) * WSLAB[0]]),
            "w123s": np.concatenate(
                [wcat[1][c * WSLAB[1]:(c + 1) * WSLAB[1]],
                 wcat[2][c * WSLAB[2]:(c + 1) * WSLAB[2]],
                 wcat[3][c * WSLAB[3]:(c + 1) * WSLAB[3]]], axis=0),
        }
        in_maps.append(m)
    return in_maps


_LAST_RESULTS = {}


def kernel(**inputs):
    h0, G, W = _host_prep(inputs)
    s = float(np.abs(h0).max() / 2047.0)
    nc, SLOTS = build_program(G, W["npos"], float(np.asarray(inputs["bh"])[0]),
                              s)
    in_maps = make_in_maps(h0, G, W)
    res = run_bass_kernel_spmd(nc, in_maps, core_ids=list(range(NC)))
    _LAST_RESULTS["res"] = res
    pred = np.concatenate(
        [res.results[c]["pred"].reshape(-1)[:NPC] for c in range(NC)])
    ti = np.asarray(inputs["train_idx"])
    return pred[ti].astype(np.float32), np.asarray(inputs["y"])[ti]
